# revision 12
# baseline (speedup 1.0000x reference)
"""Trainium2 Bass kernel: linear attention (softmax over feature dim) MHA.

Math (per batch m, head h):
    q = softmax_d(xq @ Wq) * D**-0.5 ; k = softmax_d(xk @ Wk) ; v = xv @ Wv
    kv_h = k_h^T @ v_h            [d, e]
    out_h = q_h @ kv_h            [n, e]
    out = concat_h(out_h) @ Wo + bo

Sharding: data-parallel over batch m (16 batches -> 2 per core, 8 cores).
No collectives. Host-side marshalling: per-core shards are uploaded as
bf16, with x tensors pre-transposed to [batch, d_model, n] so every
matmul contraction sits on the SBUF partition axis.

Device pipeline per (core, batch):
  pass 1 (tokens in chunks of 128):
    psum_k = xkT^T @ Wk            (k in natural [tok, f] layout)
    ke     = exp(psum_k)           -> bf16 SBUF      (ACT)
    s_k    = rowsum per head       (DVE segmented reduce)
    rk     = 1/s_k                 (DVE)
    psum_v = xvT^T @ Wv
    vs     = psum_v * rk[bcast]    -> bf16 SBUF      (DVE, k-softmax folded into v)
    kv_g  += ke_h^T @ vs_h         (PE, head pairs packed into 128 partitions
                                    via tile_position col groups)
  kvblk_g = block-diag([kv_2g, kv_2g+1])  -> bf16 SBUF
  pass 2 (tokens in chunks of 512):
    psum_q = Wq^T @ xqT            (q in transposed [f, tok] layout)
    qe     = exp(psum_q)           -> bf16
    s_q   += pool8^T @ qe          (PE partition-pooling matmul)
    rq     = 1/s_q
    o5     = kvblk^T @ qe          (PSUM, = (exp(q) @ kv)^T per head pair)
    bc     = exp8^T @ rq           (PE broadcast of rq across partitions)
    o5n    = o5 * bc               -> bf16 (q-softmax denominator applied)
    psum_o = o5n^T @ Wo_scaled     (scale = D**-0.5 folded into Wo on host)
    out    = copy(psum_o) -> f32 -> DRAM
bo is added on the host (it is tiny); output returned as f32.
"""

import os
import sys

for _p in ("/opt/trn_rl_repo", "/root/.axon_site/_ro/trn_rl_repo"):
    if os.path.isdir(_p) and _p not in sys.path:
        sys.path.insert(0, _p)

from contextlib import ExitStack

import ml_dtypes
import numpy as np

import concourse.mybir as mybir
import concourse.tile as tile
from concourse import bacc
from concourse.bass import ds, ts
from concourse.bass_utils import run_bass_kernel_spmd

BF16 = mybir.dt.bfloat16
F32 = mybir.dt.float32
F32R = mybir.dt.float32r
NPBF16 = ml_dtypes.bfloat16

M, N, DM = 16, 2048, 512
H, D = 8, 64
NCORES = 8
MB = M // NCORES          # batches per core
NC_DM = DM // 128         # 4 contraction chunks of 128
NT128 = N // 128          # 16 token chunks (pass 1)
NT512 = N // 512          # 4 token chunks (pass 2)
NPAIR = H // 2            # 4 head pairs

EXP = mybir.ActivationFunctionType.Exp
COPY = mybir.ActivationFunctionType.Copy


def build_program(reps: int = 1, loop_n: int = 1):
    nc = bacc.Bacc(
        "TRN2", target_bir_lowering=False, debug=False, num_devices=NCORES
    )
    xqT = nc.dram_tensor("xqT", [MB, DM, N], BF16, kind="ExternalInput").ap()
    xkT = nc.dram_tensor("xkT", [MB, DM, N], BF16, kind="ExternalInput").ap()
    xvT = nc.dram_tensor("xvT", [MB, DM, N], BF16, kind="ExternalInput").ap()
    w_dram = {
        name: nc.dram_tensor(name, [DM, DM], BF16, kind="ExternalInput").ap()
        for name in ("wq", "wk", "wv", "wo")
    }
    # pool8[p, c, h] = 1 iff h == 2c + p//64 : per-head partition pooling
    pool8_d = nc.dram_tensor("pool8", [128, NC_DM, H], BF16, kind="ExternalInput").ap()
    # exp8[h, 128c + j] = 1 iff h == 2c + j//64 : partition broadcast
    exp8_d = nc.dram_tensor("exp8", [H, DM], F32R, kind="ExternalInput").ap()
    out_d = nc.dram_tensor("out", [MB, N, DM], F32, kind="ExternalOutput").ap()

    with tile.TileContext(nc) as tc, ExitStack() as ctx:
        wpool = ctx.enter_context(tc.tile_pool(name="w", bufs=1))
        xpool = ctx.enter_context(tc.tile_pool(name="x", bufs=2))
        kepool = ctx.enter_context(tc.tile_pool(name="ke", bufs=4))
        vspool = ctx.enter_context(tc.tile_pool(name="vs", bufs=4))
        skpool = ctx.enter_context(tc.tile_pool(name="sk", bufs=6))
        kbpool = ctx.enter_context(tc.tile_pool(name="kvblk", bufs=8))
        qepool = ctx.enter_context(tc.tile_pool(name="qe", bufs=6))
        rqpool = ctx.enter_context(tc.tile_pool(name="rq", bufs=2))
        o5pool = ctx.enter_context(tc.tile_pool(name="o5", bufs=6))
        fpool = ctx.enter_context(tc.tile_pool(name="fin", bufs=3))
        ps_mm = ctx.enter_context(tc.tile_pool(name="ps_mm", bufs=3, space="PSUM"))
        ps_kv = ctx.enter_context(tc.tile_pool(name="ps_kv", bufs=4, space="PSUM"))
        ps_s = ctx.enter_context(tc.tile_pool(name="ps_s", bufs=1, space="PSUM"))

        w_sb = {}
        for name in ("wq", "wk", "wv", "wo"):
            t = wpool.tile([128, NC_DM, DM], BF16, tag=name)
            nc.sync.dma_start(
                out=t[:, :, :],
                in_=w_dram[name].rearrange("(c p) f -> p c f", p=128),
            )
            w_sb[name] = t
        pool8_sb = wpool.tile([128, NC_DM, H], BF16, tag="pool8")
        nc.sync.dma_start(out=pool8_sb[:, :, :], in_=pool8_d)
        exp8_sb = wpool.tile([H, DM], F32R, tag="exp8")
        nc.sync.dma_start(out=exp8_sb[:, :], in_=exp8_d)

        loop_ctx = tc.For_i(0, loop_n, 1) if loop_n > 1 else None
        if loop_ctx is not None:
            ctx.enter_context(loop_ctx)
        for _rep in range(reps):
            for b in range(MB):
                xk_sb = xpool.tile([128, NC_DM, N], BF16, tag="xk")
                nc.sync.dma_start(
                    out=xk_sb[:, :, :],
                    in_=xkT[b].rearrange("(c p) n -> p c n", p=128),
                )
                xv_sb = xpool.tile([128, NC_DM, N], BF16, tag="xv")
                nc.sync.dma_start(
                    out=xv_sb[:, :, :],
                    in_=xvT[b].rearrange("(c p) n -> p c n", p=128),
                )
                xq_sb = xpool.tile([128, NC_DM, N], BF16, tag="xq")
                nc.sync.dma_start(
                    out=xq_sb[:, :, :],
                    in_=xqT[b].rearrange("(c p) n -> p c n", p=128),
                )

                # ---------------- pass 1: kv = k_sm^T v --------------------
                kv_ps = [
                    ps_kv.tile([128, 128], F32, tag="kv", name=f"kv{g}")
                    for g in range(NPAIR)
                ]
                for tk in range(NT128):
                    ps_k = ps_mm.tile([128, DM], F32, tag="mm512")
                    for c in range(NC_DM):
                        nc.tensor.matmul(
                            ps_k[:, :],
                            xk_sb[:, c, ts(tk, 128)],
                            w_sb["wk"][:, c, :],
                            start=(c == 0),
                            stop=(c == NC_DM - 1),
                        )
                    ke = kepool.tile([128, H, D], BF16, tag="ke")
                    nc.scalar.activation(
                        ke[:, :, :],
                        ps_k[:, :].rearrange("p (h e) -> p h e", h=H),
                        EXP,
                    )
                    sk = skpool.tile([128, H], F32, tag="sk")
                    nc.vector.tensor_reduce(
                        sk[:, :],
                        ke[:, :, :],
                        axis=mybir.AxisListType.X,
                        op=mybir.AluOpType.add,
                    )
                    rk = skpool.tile([128, H], F32, tag="rk")
                    nc.vector.reciprocal(rk[:, :], sk[:, :])

                    ps_v = ps_mm.tile([128, DM], F32, tag="mm512")
                    for c in range(NC_DM):
                        nc.tensor.matmul(
                            ps_v[:, :],
                            xv_sb[:, c, ts(tk, 128)],
                            w_sb["wv"][:, c, :],
                            start=(c == 0),
                            stop=(c == NC_DM - 1),
                        )
                    vs = vspool.tile([128, H, D], BF16, tag="vs")
                    nc.vector.tensor_mul(
                        vs[:, :, :],
                        ps_v[:, :].rearrange("p (h e) -> p h e", h=H),
                        rk[:, :].to_broadcast([128, H, D]),
                    )
                    for g in range(NPAIR):
                        # 2-head pack: stat/mov [128, 128]; off-diagonal
                        # cross-head blocks are garbage, dropped below.
                        nc.tensor.matmul(
                            kv_ps[g][:, :],
                            ke[:, ds(2 * g, 2), :],
                            vs[:, ds(2 * g, 2), :],
                            start=(tk == 0),
                            stop=(tk == NT128 - 1),
                        )

                kvblks = []
                for g in range(NPAIR):
                    kb = kbpool.tile([128, 128], BF16, tag="kvblk")
                    nc.vector.memset(kb[:, :], 0.0)
                    nc.vector.tensor_copy(kb[0:64, 0:64], kv_ps[g][0:64, 0:64])
                    nc.vector.tensor_copy(kb[64:128, 64:128], kv_ps[g][64:128, 64:128])
                    kvblks.append(kb)

                # ---------------- pass 2: out = (q_sm @ kv) @ Wo -----------
                for t in range(NT512):
                    s_ps = ps_s.tile([H, 512], F32, tag="s")
                    qes = []
                    for c in range(NC_DM):
                        ps_q = ps_mm.tile([128, 512], F32, tag="mm512")
                        for k in range(NC_DM):
                            nc.tensor.matmul(
                                ps_q[:, :],
                                w_sb["wq"][:, k, ds(128 * c, 128)],
                                xq_sb[:, k, ds(512 * t, 512)],
                                start=(k == 0),
                                stop=(k == NC_DM - 1),
                            )
                        qe = qepool.tile([128, 512], BF16, tag="qe")
                        nc.scalar.activation(qe[:, :], ps_q[:, :], EXP)
                        nc.tensor.matmul(
                            s_ps[:, :],
                            pool8_sb[:, c, :],
                            qe[:, :],
                            start=(c == 0),
                            stop=(c == NC_DM - 1),
                        )
                        qes.append(qe)
                    rq32 = rqpool.tile([H, 512], F32, tag="rq32")
                    nc.vector.reciprocal_approx_fast(rq32[:, :], s_ps[:, :])
                    rq = rqpool.tile([H, 512], F32R, tag="rq")
                    nc.vector.tensor_copy(rq[:, :], rq32[:, :])

                    o5s = []
                    for c in range(NC_DM):
                        o5 = ps_mm.tile([128, 512], F32, tag="mm512")
                        nc.tensor.matmul(
                            o5[:, :], kvblks[c][:, :], qes[c][:, :],
                            start=True, stop=True,
                        )
                        bc = ps_kv.tile([128, 512], F32, tag="kv")
                        nc.tensor.matmul(
                            bc[:, :],
                            exp8_sb[:, ds(128 * c, 128)],
                            rq[:, :],
                            start=True, stop=True,
                        )
                        o5c = o5pool.tile([128, 512], BF16, tag="o5c")
                        nc.scalar.activation(o5c[:, :], o5[:, :], COPY)
                        o5n = o5pool.tile([128, 512], BF16, tag="o5")
                        nc.vector.tensor_mul(o5n[:, :], o5c[:, :], bc[:, :])
                        o5s.append(o5n)

                    for u in range(4):
                        ps_o = ps_mm.tile([128, DM], F32, tag="mm512")
                        for c in range(NC_DM):
                            nc.tensor.matmul(
                                ps_o[:, :],
                                o5s[c][:, ds(128 * u, 128)],
                                w_sb["wo"][:, c, :],
                                start=(c == 0),
                                stop=(c == NC_DM - 1),
                            )
                        fin = fpool.tile([128, DM], F32, tag="fin")
                        nc.scalar.activation(fin[:, :], ps_o[:, :], COPY)
                        nc.sync.dma_start(
                            out=out_d[b, ds(512 * t + 128 * u, 128), :],
                            in_=fin[:, :],
                        )
    nc.compile()
    return nc


def make_const_inputs():
    pool8 = np.zeros((128, NC_DM, H), np.float32)
    for p in range(128):
        for c in range(NC_DM):
            pool8[p, c, 2 * c + p // 64] = 1.0
    exp8 = np.zeros((H, DM), np.float32)
    for c in range(NC_DM):
        for j in range(128):
            exp8[2 * c + j // 64, 128 * c + j] = 1.0
    return pool8.astype(NPBF16), exp8


def make_in_maps(xq, xk, xv, Wq, Wk, Wv, Wo):
    pool8, exp8 = make_const_inputs()
    scale = np.float32(D**-0.5)
    consts = {
        "wq": np.asarray(Wq, np.float32).astype(NPBF16),
        "wk": np.asarray(Wk, np.float32).astype(NPBF16),
        "wv": np.asarray(Wv, np.float32).astype(NPBF16),
        "wo": (np.asarray(Wo, np.float32) * scale).astype(NPBF16),
        "pool8": pool8,
        "exp8": exp8,
    }

    def prep(x, sl):
        xt = np.asarray(x[sl], np.float32).transpose(0, 2, 1)
        return np.ascontiguousarray(xt).astype(NPBF16)

    in_maps = []
    for core in range(NCORES):
        sl = slice(MB * core, MB * (core + 1))
        m = dict(consts)
        m["xqT"] = prep(xq, sl)
        m["xkT"] = prep(xk, sl)
        m["xvT"] = prep(xv, sl)
        in_maps.append(m)
    return in_maps


_NC = None


def kernel(xq, xk, xv, Wq, Wk, Wv, Wo, bo):
    global _NC
    if _NC is None:
        _NC = build_program()
    in_maps = make_in_maps(xq, xk, xv, Wq, Wk, Wv, Wo)
    res = run_bass_kernel_spmd(_NC, in_maps, core_ids=list(range(NCORES)))
    out = np.concatenate([res.results[i]["out"] for i in range(NCORES)], axis=0)
    out += np.asarray(bo, np.float32)[None, None, :]
    return out


# revision 20
# speedup vs baseline: 13.7464x; 13.7464x over previous
"""Trainium2 Bass kernel: linear attention (softmax over feature dim) MHA.

Math (per batch m, head h):
    q = softmax_d(xq @ Wq) * D**-0.5 ; k = softmax_d(xk @ Wk) ; v = xv @ Wv
    kv_h = k_h^T @ v_h            [d, e]
    out_h = q_h @ kv_h            [n, e]
    out = concat_h(out_h) @ Wo + bo

Sharding: data-parallel over batch m (16 batches -> 2 per core, 8 cores).
No collectives. Host-side marshalling: per-core shards are uploaded as
bf16, with x tensors pre-transposed to [batch, d_model, n] so every
matmul contraction sits on the SBUF partition axis.

Device pipeline per (core, batch):
  pass 1 (tokens in chunks of 128):
    psum_k = xkT^T @ Wk            (k in natural [tok, f] layout)
    ke     = exp(psum_k)           -> bf16 SBUF      (ACT)
    s_k    = rowsum per head       (DVE segmented reduce)
    rk     = 1/s_k                 (DVE)
    psum_v = xvT^T @ Wv
    vs     = psum_v * rk[bcast]    -> bf16 SBUF      (DVE, k-softmax folded into v)
    kv_g  += ke_pair^T @ vs_pair   (PE, two heads packed per 128x128 matmul;
                                    cross-head blocks discarded below)
  kvblk_g = block-diag([kv_2g, kv_2g+1])  -> bf16 SBUF
  pass 2 (tokens in chunks of 512):
    psum_q = Wq^T @ xqT            (q in transposed [f, tok] layout)
    qe     = exp(psum_q)           -> bf16
    s_q   += pool8^T @ qe          (PE partition-pooling matmul)
    rq     = 1/s_q                 (DVE reciprocal_approx_fast, ~18-bit)
    o5     = kvblk^T @ qe          (PSUM, = (exp(q) @ kv)^T per head pair)
    bc     = exp8^T @ rq           (PE broadcast of rq across partitions)
    o5n    = o5 * bc               -> bf16 (q-softmax denominator applied)
    psum_o = o5n^T @ Wo_scaled     (scale = D**-0.5 folded into Wo on host)
    out    = copy(psum_o) -> f32 -> DRAM
bo is added on the host (it is tiny); output returned as f32.
"""

import os
import sys

for _p in ("/opt/trn_rl_repo", "/root/.axon_site/_ro/trn_rl_repo"):
    if os.path.isdir(_p) and _p not in sys.path:
        sys.path.insert(0, _p)

from contextlib import ExitStack

import ml_dtypes
import numpy as np

import concourse.mybir as mybir
import concourse.tile as tile
from concourse import bacc
from concourse.bass import ds, ts
from concourse.bass_utils import run_bass_kernel_spmd

BF16 = mybir.dt.bfloat16
F32 = mybir.dt.float32
F32R = mybir.dt.float32r
NPBF16 = ml_dtypes.bfloat16

M, N, DM = 16, 2048, 512
H, D = 8, 64
NCORES = 8
MB = M // NCORES          # batches per core
NC_DM = DM // 128         # 4 contraction chunks of 128
NT128 = N // 128          # 16 token chunks (pass 1)
NT512 = N // 512          # 4 token chunks (pass 2)
NPAIR = H // 2            # 4 head pairs

EXP = mybir.ActivationFunctionType.Exp
COPY = mybir.ActivationFunctionType.Copy


def build_program(reps: int = 1, loop_n: int = 1):
    nc = bacc.Bacc(
        "TRN2", target_bir_lowering=False, debug=False, num_devices=NCORES
    )
    xqT = nc.dram_tensor("xqT", [MB, DM, N], BF16, kind="ExternalInput").ap()
    xkT = nc.dram_tensor("xkT", [MB, DM, N], BF16, kind="ExternalInput").ap()
    xvT = nc.dram_tensor("xvT", [MB, DM, N], BF16, kind="ExternalInput").ap()
    w_dram = {
        name: nc.dram_tensor(name, [DM, DM], BF16, kind="ExternalInput").ap()
        for name in ("wq", "wk", "wv", "wo")
    }
    # pool8[p, c, h] = 1 iff h == 2c + p//64 : per-head partition pooling
    pool8_d = nc.dram_tensor("pool8", [128, NC_DM, H], BF16, kind="ExternalInput").ap()
    # exp8[h, 128c + j] = 1 iff h == 2c + j//64 : partition broadcast
    exp8_d = nc.dram_tensor("exp8", [H, DM], F32R, kind="ExternalInput").ap()
    out_d = nc.dram_tensor("out", [MB, N, DM], F32, kind="ExternalOutput").ap()

    with tile.TileContext(nc) as tc, ExitStack() as ctx:
        wpool = ctx.enter_context(tc.tile_pool(name="w", bufs=1))
        xpool = ctx.enter_context(tc.tile_pool(name="x", bufs=2))
        kepool = ctx.enter_context(tc.tile_pool(name="ke", bufs=6))
        vspool = ctx.enter_context(tc.tile_pool(name="vs", bufs=6))
        skpool = ctx.enter_context(tc.tile_pool(name="sk", bufs=8))
        kbpool = ctx.enter_context(tc.tile_pool(name="kvblk", bufs=8))
        qepool = ctx.enter_context(tc.tile_pool(name="qe", bufs=10))
        rqpool = ctx.enter_context(tc.tile_pool(name="rq", bufs=2))
        o5pool = ctx.enter_context(tc.tile_pool(name="o5", bufs=10))
        fpool = ctx.enter_context(tc.tile_pool(name="fin", bufs=4))
        ps_mm = ctx.enter_context(tc.tile_pool(name="ps_mm", bufs=4, space="PSUM"))
        ps_kv = ctx.enter_context(tc.tile_pool(name="ps_kv", bufs=4, space="PSUM"))

        w_sb = {}
        for name in ("wq", "wk", "wv", "wo"):
            t = wpool.tile([128, NC_DM, DM], BF16, tag=name)
            nc.sync.dma_start(
                out=t[:, :, :],
                in_=w_dram[name].rearrange("(c p) f -> p c f", p=128),
            )
            w_sb[name] = t
        pool8_sb = wpool.tile([128, NC_DM, H], BF16, tag="pool8")
        nc.sync.dma_start(out=pool8_sb[:, :, :], in_=pool8_d)
        exp8_sb = wpool.tile([H, DM], F32R, tag="exp8")
        nc.sync.dma_start(out=exp8_sb[:, :], in_=exp8_d)

        loop_ctx = tc.For_i(0, loop_n, 1) if loop_n > 1 else None
        if loop_ctx is not None:
            ctx.enter_context(loop_ctx)
        for _rep in range(reps):
            for b in range(MB):
                xk_sb = xpool.tile([128, NC_DM, N], BF16, tag="xk")
                nc.sync.dma_start(
                    out=xk_sb[:, :, :],
                    in_=xkT[b].rearrange("(c p) n -> p c n", p=128),
                )
                xv_sb = xpool.tile([128, NC_DM, N], BF16, tag="xv")
                nc.sync.dma_start(
                    out=xv_sb[:, :, :],
                    in_=xvT[b].rearrange("(c p) n -> p c n", p=128),
                )
                xq_sb = xpool.tile([128, NC_DM, N], BF16, tag="xq")
                nc.sync.dma_start(
                    out=xq_sb[:, :, :],
                    in_=xqT[b].rearrange("(c p) n -> p c n", p=128),
                )

                # ---------------- pass 1: kv = k_sm^T v --------------------
                kv_ps = [
                    ps_kv.tile([128, 128], F32, tag="kv", name=f"kv{g}")
                    for g in range(NPAIR)
                ]
                for tk in range(NT128):
                    ps_k = ps_mm.tile([128, DM], F32, tag="mm512")
                    for c in range(NC_DM):
                        nc.tensor.matmul(
                            ps_k[:, :],
                            xk_sb[:, c, ts(tk, 128)],
                            w_sb["wk"][:, c, :],
                            start=(c == 0),
                            stop=(c == NC_DM - 1),
                        )
                    ke = kepool.tile([128, H, D], BF16, tag="ke")
                    nc.scalar.activation(
                        ke[:, :, :],
                        ps_k[:, :].rearrange("p (h e) -> p h e", h=H),
                        EXP,
                    )
                    sk = skpool.tile([128, H], F32, tag="sk")
                    nc.vector.tensor_reduce(
                        sk[:, :],
                        ke[:, :, :],
                        axis=mybir.AxisListType.X,
                        op=mybir.AluOpType.add,
                    )
                    rk = skpool.tile([128, H], F32, tag="rk")
                    nc.vector.reciprocal(rk[:, :], sk[:, :])

                    ps_v = ps_mm.tile([128, DM], F32, tag="mm512")
                    for c in range(NC_DM):
                        nc.tensor.matmul(
                            ps_v[:, :],
                            xv_sb[:, c, ts(tk, 128)],
                            w_sb["wv"][:, c, :],
                            start=(c == 0),
                            stop=(c == NC_DM - 1),
                        )
                    vs = vspool.tile([128, H, D], BF16, tag="vs")
                    nc.vector.tensor_mul(
                        vs[:, :, :],
                        ps_v[:, :].rearrange("p (h e) -> p h e", h=H),
                        rk[:, :].to_broadcast([128, H, D]),
                    )
                    for g in range(NPAIR):
                        # 2-head pack: stat/mov [128, 128]; off-diagonal
                        # cross-head blocks are garbage, dropped below.
                        nc.tensor.matmul(
                            kv_ps[g][:, :],
                            ke[:, ds(2 * g, 2), :],
                            vs[:, ds(2 * g, 2), :],
                            start=(tk == 0),
                            stop=(tk == NT128 - 1),
                        )

                kvblks = []
                for g in range(NPAIR):
                    kb = kbpool.tile([128, 128], BF16, tag="kvblk")
                    nc.vector.memset(kb[:, :], 0.0)
                    nc.vector.tensor_copy(kb[0:64, 0:64], kv_ps[g][0:64, 0:64])
                    nc.vector.tensor_copy(kb[64:128, 64:128], kv_ps[g][64:128, 64:128])
                    kvblks.append(kb)

                # ---------------- pass 2: out = (q_sm @ kv) @ Wo -----------
                for t in range(NT512):
                    s_ps = ps_kv.tile([H, 512], F32, tag="kv", name="s_ps")
                    qes = []
                    for c in range(NC_DM):
                        ps_q = ps_mm.tile([128, 512], F32, tag="mm512")
                        for k in range(NC_DM):
                            nc.tensor.matmul(
                                ps_q[:, :],
                                w_sb["wq"][:, k, ds(128 * c, 128)],
                                xq_sb[:, k, ds(512 * t, 512)],
                                start=(k == 0),
                                stop=(k == NC_DM - 1),
                            )
                        qe = qepool.tile([128, 512], BF16, tag="qe")
                        nc.scalar.activation(qe[:, :], ps_q[:, :], EXP)
                        nc.tensor.matmul(
                            s_ps[:, :],
                            pool8_sb[:, c, :],
                            qe[:, :],
                            start=(c == 0),
                            stop=(c == NC_DM - 1),
                        )
                        qes.append(qe)
                    rq32 = rqpool.tile([H, 512], F32, tag="rq32")
                    nc.vector.reciprocal_approx_fast(rq32[:, :], s_ps[:, :])
                    rq = rqpool.tile([H, 512], F32R, tag="rq")
                    nc.vector.tensor_copy(rq[:, :], rq32[:, :])

                    o5s = []
                    for c in range(NC_DM):
                        o5 = ps_mm.tile([128, 512], F32, tag="mm512")
                        nc.tensor.matmul(
                            o5[:, :], kvblks[c][:, :], qes[c][:, :],
                            start=True, stop=True,
                        )
                        bc = ps_kv.tile([128, 512], F32, tag="kv")
                        nc.tensor.matmul(
                            bc[:, :],
                            exp8_sb[:, ds(128 * c, 128)],
                            rq[:, :],
                            start=True, stop=True,
                        )
                        o5c = o5pool.tile([128, 512], BF16, tag="o5c")
                        nc.scalar.activation(o5c[:, :], o5[:, :], COPY)
                        o5n = o5pool.tile([128, 512], BF16, tag="o5")
                        nc.vector.tensor_mul(o5n[:, :], o5c[:, :], bc[:, :])
                        o5s.append(o5n)

                    for u in range(4):
                        ps_o = ps_mm.tile([128, DM], F32, tag="mm512")
                        for c in range(NC_DM):
                            nc.tensor.matmul(
                                ps_o[:, :],
                                o5s[c][:, ds(128 * u, 128)],
                                w_sb["wo"][:, c, :],
                                start=(c == 0),
                                stop=(c == NC_DM - 1),
                            )
                        fin = fpool.tile([128, DM], F32, tag="fin")
                        nc.scalar.activation(fin[:, :], ps_o[:, :], COPY)
                        nc.sync.dma_start(
                            out=out_d[b, ds(512 * t + 128 * u, 128), :],
                            in_=fin[:, :],
                        )
    nc.compile()
    return nc


def make_const_inputs():
    pool8 = np.zeros((128, NC_DM, H), np.float32)
    for p in range(128):
        for c in range(NC_DM):
            pool8[p, c, 2 * c + p // 64] = 1.0
    exp8 = np.zeros((H, DM), np.float32)
    for c in range(NC_DM):
        for j in range(128):
            exp8[2 * c + j // 64, 128 * c + j] = 1.0
    return pool8.astype(NPBF16), exp8


def make_in_maps(xq, xk, xv, Wq, Wk, Wv, Wo):
    pool8, exp8 = make_const_inputs()
    scale = np.float32(D**-0.5)
    consts = {
        "wq": np.asarray(Wq, np.float32).astype(NPBF16),
        "wk": np.asarray(Wk, np.float32).astype(NPBF16),
        "wv": np.asarray(Wv, np.float32).astype(NPBF16),
        "wo": (np.asarray(Wo, np.float32) * scale).astype(NPBF16),
        "pool8": pool8,
        "exp8": exp8,
    }

    def prep(x, sl):
        xt = np.asarray(x[sl], np.float32).transpose(0, 2, 1)
        return np.ascontiguousarray(xt).astype(NPBF16)

    in_maps = []
    for core in range(NCORES):
        sl = slice(MB * core, MB * (core + 1))
        m = dict(consts)
        m["xqT"] = prep(xq, sl)
        m["xkT"] = prep(xk, sl)
        m["xvT"] = prep(xv, sl)
        in_maps.append(m)
    return in_maps


_NC = None


def kernel(xq, xk, xv, Wq, Wk, Wv, Wo, bo):
    global _NC
    if _NC is None:
        _NC = build_program()
    in_maps = make_in_maps(xq, xk, xv, Wq, Wk, Wv, Wo)
    res = run_bass_kernel_spmd(_NC, in_maps, core_ids=list(range(NCORES)))
    out = np.concatenate([res.results[i]["out"] for i in range(NCORES)], axis=0)
    out += np.asarray(bo, np.float32)[None, None, :]
    return out


# revision 22
# speedup vs baseline: 13.9452x; 1.0145x over previous
"""Trainium2 Bass kernel: linear attention (softmax over feature dim) MHA.

Math (per batch m, head h):
    q = softmax_d(xq @ Wq) * D**-0.5 ; k = softmax_d(xk @ Wk) ; v = xv @ Wv
    kv_h = k_h^T @ v_h            [d, e]
    out_h = q_h @ kv_h            [n, e]
    out = concat_h(out_h) @ Wo + bo

Sharding: data-parallel over batch m (16 batches -> 2 per core, 8 cores).
No collectives. Host-side marshalling: per-core shards are uploaded as
bf16, with x tensors pre-transposed to [batch, d_model, n] so every
matmul contraction sits on the SBUF partition axis.

Device pipeline per (core, batch):
  pass 1 (tokens in chunks of 128):
    psum_k = xkT^T @ Wk            (k in natural [tok, f] layout)
    ke     = exp(psum_k)           -> bf16 SBUF      (ACT)
    s_k    = rowsum per head       (DVE segmented reduce)
    rk     = 1/s_k                 (DVE)
    psum_v = xvT^T @ Wv
    vs     = psum_v * rk[bcast]    -> bf16 SBUF      (DVE, k-softmax folded into v)
    kv_g  += ke_h^T @ vs_h         (PE, head pairs packed into 128 partitions
                                    via tile_position col groups)
  kvblk_g = block-diag([kv_2g, kv_2g+1])  -> bf16 SBUF
  pass 2 (tokens in chunks of 512):
    psum_q = Wq^T @ xqT            (q in transposed [f, tok] layout)
    qe     = exp(psum_q)           -> bf16
    s_q   += pool8^T @ qe          (PE partition-pooling matmul)
    rq     = 1/s_q
    o5     = kvblk^T @ qe          (PSUM, = (exp(q) @ kv)^T per head pair)
    bc     = exp8^T @ rq           (PE broadcast of rq across partitions)
    o5n    = o5 * bc               -> bf16 (q-softmax denominator applied)
    psum_o = o5n^T @ Wo_scaled     (scale = D**-0.5 folded into Wo on host)
    out    = copy(psum_o) -> f32 -> DRAM
bo is added on the host (it is tiny); output returned as f32.
"""

import os
import sys

for _p in ("/opt/trn_rl_repo", "/root/.axon_site/_ro/trn_rl_repo"):
    if os.path.isdir(_p) and _p not in sys.path:
        sys.path.insert(0, _p)

from contextlib import ExitStack

import ml_dtypes
import numpy as np

import concourse.mybir as mybir
import concourse.tile as tile
from concourse import bacc
from concourse.bass import ds, ts
from concourse.bass_utils import run_bass_kernel_spmd

BF16 = mybir.dt.bfloat16
F32 = mybir.dt.float32
F32R = mybir.dt.float32r
NPBF16 = ml_dtypes.bfloat16

M, N, DM = 16, 2048, 512
H, D = 8, 64
NCORES = 8
MB = M // NCORES          # batches per core
NC_DM = DM // 128         # 4 contraction chunks of 128
NT128 = N // 128          # 16 token chunks (pass 1)
NT512 = N // 512          # 4 token chunks (pass 2)
NPAIR = H // 2            # 4 head pairs

EXP = mybir.ActivationFunctionType.Exp
COPY = mybir.ActivationFunctionType.Copy


def build_program(reps: int = 1, loop_n: int = 1):
    nc = bacc.Bacc(
        "TRN2", target_bir_lowering=False, debug=False, num_devices=NCORES
    )
    xqT = nc.dram_tensor("xqT", [MB, DM, N], BF16, kind="ExternalInput").ap()
    xkT = nc.dram_tensor("xkT", [MB, DM, N], BF16, kind="ExternalInput").ap()
    xvT = nc.dram_tensor("xvT", [MB, DM, N], BF16, kind="ExternalInput").ap()
    w_dram = {
        name: nc.dram_tensor(name, [DM, DM], BF16, kind="ExternalInput").ap()
        for name in ("wq", "wk", "wv", "wo")
    }
    # pool8[p, c, h] = 1 iff h == 2c + p//64 : per-head partition pooling
    pool8_d = nc.dram_tensor("pool8", [128, NC_DM, H], BF16, kind="ExternalInput").ap()
    # exp8[h, 128c + j] = 1 iff h == 2c + j//64 : partition broadcast
    exp8_d = nc.dram_tensor("exp8", [H, DM], F32R, kind="ExternalInput").ap()
    out_d = nc.dram_tensor("out", [MB, N, DM], F32, kind="ExternalOutput").ap()

    with tile.TileContext(nc) as tc, ExitStack() as ctx:
        wpool = ctx.enter_context(tc.tile_pool(name="w", bufs=1))
        xpool = ctx.enter_context(tc.tile_pool(name="x", bufs=2))
        kepool = ctx.enter_context(tc.tile_pool(name="ke", bufs=6))
        vspool = ctx.enter_context(tc.tile_pool(name="vs", bufs=6))
        skpool = ctx.enter_context(tc.tile_pool(name="sk", bufs=8))
        kbpool = ctx.enter_context(tc.tile_pool(name="kvblk", bufs=8))
        qepool = ctx.enter_context(tc.tile_pool(name="qe", bufs=10))
        rqpool = ctx.enter_context(tc.tile_pool(name="rq", bufs=2))
        o5pool = ctx.enter_context(tc.tile_pool(name="o5", bufs=10))
        fpool = ctx.enter_context(tc.tile_pool(name="fin", bufs=4))
        ps_mm = ctx.enter_context(tc.tile_pool(name="ps_mm", bufs=4, space="PSUM"))
        ps_kv = ctx.enter_context(tc.tile_pool(name="ps_kv", bufs=4, space="PSUM"))

        w_sb = {}
        for name in ("wq", "wk", "wv", "wo"):
            t = wpool.tile([128, NC_DM, DM], BF16, tag=name)
            nc.sync.dma_start(
                out=t[:, :, :],
                in_=w_dram[name].rearrange("(c p) f -> p c f", p=128),
            )
            w_sb[name] = t
        pool8_sb = wpool.tile([128, NC_DM, H], BF16, tag="pool8")
        nc.sync.dma_start(out=pool8_sb[:, :, :], in_=pool8_d)
        exp8_sb = wpool.tile([H, DM], F32R, tag="exp8")
        nc.sync.dma_start(out=exp8_sb[:, :], in_=exp8_d)

        loop_ctx = tc.For_i(0, loop_n, 1) if loop_n > 1 else None
        if loop_ctx is not None:
            ctx.enter_context(loop_ctx)
        for _rep in range(reps):
            for b in range(MB):
                xk_sb = xpool.tile([128, NC_DM, N], BF16, tag="xk")
                nc.sync.dma_start(
                    out=xk_sb[:, :, :],
                    in_=xkT[b].rearrange("(c p) n -> p c n", p=128),
                )
                xv_sb = xpool.tile([128, NC_DM, N], BF16, tag="xv")
                nc.sync.dma_start(
                    out=xv_sb[:, :, :],
                    in_=xvT[b].rearrange("(c p) n -> p c n", p=128),
                )
                xq_sb = xpool.tile([128, NC_DM, N], BF16, tag="xq")
                nc.sync.dma_start(
                    out=xq_sb[:, :, :],
                    in_=xqT[b].rearrange("(c p) n -> p c n", p=128),
                )

                # ---------------- pass 1: kv = k_sm^T v --------------------
                kv_ps = [
                    ps_kv.tile([128, 128], F32, tag="kv", name=f"kv{g}")
                    for g in range(NPAIR)
                ]
                for tk in range(NT128):
                    ps_k = ps_mm.tile([128, DM], F32, tag="mm512")
                    for c in range(NC_DM):
                        nc.tensor.matmul(
                            ps_k[:, :],
                            xk_sb[:, c, ts(tk, 128)],
                            w_sb["wk"][:, c, :],
                            start=(c == 0),
                            stop=(c == NC_DM - 1),
                        )
                    ke = kepool.tile([128, H, D], BF16, tag="ke")
                    nc.scalar.activation(
                        ke[:, :, :],
                        ps_k[:, :].rearrange("p (h e) -> p h e", h=H),
                        EXP,
                    )
                    sk = skpool.tile([128, H], F32, tag="sk")
                    nc.vector.tensor_reduce(
                        sk[:, :],
                        ke[:, :, :],
                        axis=mybir.AxisListType.X,
                        op=mybir.AluOpType.add,
                    )
                    rk = skpool.tile([128, H], F32, tag="rk")
                    nc.vector.reciprocal(rk[:, :], sk[:, :])

                    ps_v = ps_mm.tile([128, DM], F32, tag="mm512")
                    for c in range(NC_DM):
                        nc.tensor.matmul(
                            ps_v[:, :],
                            xv_sb[:, c, ts(tk, 128)],
                            w_sb["wv"][:, c, :],
                            start=(c == 0),
                            stop=(c == NC_DM - 1),
                        )
                    vs = vspool.tile([128, H, D], BF16, tag="vs")
                    nc.vector.tensor_mul(
                        vs[:, :, :],
                        ps_v[:, :].rearrange("p (h e) -> p h e", h=H),
                        rk[:, :].to_broadcast([128, H, D]),
                    )
                    for g in range(NPAIR):
                        # 2-head pack: stat/mov [128, 128]; off-diagonal
                        # cross-head blocks are garbage, dropped below.
                        nc.tensor.matmul(
                            kv_ps[g][:, :],
                            ke[:, ds(2 * g, 2), :],
                            vs[:, ds(2 * g, 2), :],
                            start=(tk == 0),
                            stop=(tk == NT128 - 1),
                        )

                kvblks = []
                for g in range(NPAIR):
                    kb = kbpool.tile([128, 128], BF16, tag="kvblk")
                    nc.vector.memset(kb[:, :], 0.0)
                    nc.vector.tensor_copy(kb[0:64, 0:64], kv_ps[g][0:64, 0:64])
                    nc.vector.tensor_copy(kb[64:128, 64:128], kv_ps[g][64:128, 64:128])
                    kvblks.append(kb)

                # ---------------- pass 2: out = (q_sm @ kv) @ Wo -----------
                for t in range(NT512):
                    s_ps = ps_kv.tile([H, 512], F32, tag="kv", name="s_ps")
                    qes = []
                    for c in range(NC_DM):
                        ps_q = ps_mm.tile([128, 512], F32, tag="mm512")
                        for k in range(NC_DM):
                            nc.tensor.matmul(
                                ps_q[:, :],
                                w_sb["wq"][:, k, ds(128 * c, 128)],
                                xq_sb[:, k, ds(512 * t, 512)],
                                start=(k == 0),
                                stop=(k == NC_DM - 1),
                            )
                        qe = qepool.tile([128, 512], BF16, tag="qe")
                        nc.scalar.activation(qe[:, :], ps_q[:, :], EXP)
                        nc.tensor.matmul(
                            s_ps[:, :],
                            pool8_sb[:, c, :],
                            qe[:, :],
                            start=(c == 0),
                            stop=(c == NC_DM - 1),
                        )
                        qes.append(qe)
                    rq32 = rqpool.tile([H, 512], F32, tag="rq32")
                    nc.vector.reciprocal_approx_fast(rq32[:, :], s_ps[:, :])
                    rq = rqpool.tile([H, 512], F32R, tag="rq")
                    nc.vector.tensor_copy(rq[:, :], rq32[:, :])

                    o5s = []
                    for c in range(NC_DM):
                        o5 = ps_mm.tile([128, 512], F32, tag="mm512")
                        nc.tensor.matmul(
                            o5[:, :], kvblks[c][:, :], qes[c][:, :],
                            start=True, stop=True,
                        )
                        bc = ps_kv.tile([128, 512], F32, tag="kv")
                        nc.tensor.matmul(
                            bc[:, :],
                            exp8_sb[:, ds(128 * c, 128)],
                            rq[:, :],
                            start=True, stop=True,
                        )
                        o5c = o5pool.tile([128, 512], BF16, tag="o5c")
                        nc.scalar.activation(o5c[:, :], o5[:, :], COPY)
                        o5n = o5pool.tile([128, 512], BF16, tag="o5")
                        nc.vector.tensor_mul(o5n[:, :], o5c[:, :], bc[:, :])
                        o5s.append(o5n)

                    for u in range(4):
                        ps_o = ps_mm.tile([128, DM], F32, tag="mm512")
                        for c in range(NC_DM):
                            nc.tensor.matmul(
                                ps_o[:, :],
                                o5s[c][:, ds(128 * u, 128)],
                                w_sb["wo"][:, c, :],
                                start=(c == 0),
                                stop=(c == NC_DM - 1),
                            )
                        fin = fpool.tile([128, DM], F32, tag="fin")
                        nc.scalar.activation(fin[:, :], ps_o[:, :], COPY)
                        nc.sync.dma_start(
                            out=out_d[b, ds(512 * t + 128 * u, 128), :],
                            in_=fin[:, :],
                        )
    nc.compile()
    return nc


def make_const_inputs():
    pool8 = np.zeros((128, NC_DM, H), np.float32)
    for p in range(128):
        for c in range(NC_DM):
            pool8[p, c, 2 * c + p // 64] = 1.0
    exp8 = np.zeros((H, DM), np.float32)
    for c in range(NC_DM):
        for j in range(128):
            exp8[2 * c + j // 64, 128 * c + j] = 1.0
    return pool8.astype(NPBF16), exp8


def make_in_maps(xq, xk, xv, Wq, Wk, Wv, Wo):
    pool8, exp8 = make_const_inputs()
    scale = np.float32(D**-0.5)
    consts = {
        "wq": np.asarray(Wq, np.float32).astype(NPBF16),
        "wk": np.asarray(Wk, np.float32).astype(NPBF16),
        "wv": np.asarray(Wv, np.float32).astype(NPBF16),
        "wo": (np.asarray(Wo, np.float32) * scale).astype(NPBF16),
        "pool8": pool8,
        "exp8": exp8,
    }

    def prep(x, sl):
        xt = np.asarray(x[sl], np.float32).transpose(0, 2, 1)
        return np.ascontiguousarray(xt).astype(NPBF16)

    in_maps = []
    for core in range(NCORES):
        sl = slice(MB * core, MB * (core + 1))
        m = dict(consts)
        m["xqT"] = prep(xq, sl)
        m["xkT"] = prep(xk, sl)
        m["xvT"] = prep(xv, sl)
        in_maps.append(m)
    return in_maps


_NC = None


def kernel(xq, xk, xv, Wq, Wk, Wv, Wo, bo):
    global _NC
    if _NC is None:
        _NC = build_program()
    in_maps = make_in_maps(xq, xk, xv, Wq, Wk, Wv, Wo)
    res = run_bass_kernel_spmd(_NC, in_maps, core_ids=list(range(NCORES)))
    out = np.concatenate([res.results[i]["out"] for i in range(NCORES)], axis=0)
    out += np.asarray(bo, np.float32)[None, None, :]
    return out
